# revision 62
# baseline (speedup 1.0000x reference)
"""BalancedPrototypeLoss on 8 Trainium2 NeuronCores — v4.

Strategy (data-parallel over batch, band-parallel over prototypes):
  - similarities [16384,100,10] are cast to fp16 on host, reorganized to
    slot-major [128, NT, 10, 100] per core, and streamed over HWDGE in
    asymmetric groups (GROUPS tiles each) so the Vector engine starts early.
  - per group, the slot-max is a batched fp16 tensor_tensor max tree
    (2x DVE mode; tensor_reduce would be capped at 1x): 1000 -> 500 -> 200
    -> 100 per tile, all tiles of a group in one instruction.
  - for the leading groups: smax plus a -128 one-hot (int8 upload, SWDGE
    casts to fp16) is group-max-reduced for the separation term; per-class
    sums of smax and [sep | 1] accumulate on the TensorEngine as one-hot
    matmuls in two PSUM banks, copied out mid-kernel so the DRAM-write
    receipts hide under later compute.
  - the trailing HOST_G groups ship their raw smax to the host, which adds
    their cluster/sep/count contributions directly — the device tail is just
    the last max tree plus one DMA (no j2/relu/matmul serial chain).
  - prototype Gram: host normalizes+transposes prototypes; each core computes
    its 128-row slice against a 140-wide same-class band plus one column
    against colsum(pn) (exact Gram row sums) via 2 matmuls.
  - host combines per-core partials in float32.
"""

import sys

_TRN_REPO = "/opt/trn_rl_repo"
if _TRN_REPO not in sys.path:
    sys.path.insert(0, _TRN_REPO)

import numpy as np

import concourse.bacc as bacc
import concourse.mybir as mybir
from concourse import tile
from concourse.bass_utils import run_bass_kernel_spmd

fp32 = mybir.dt.float32
fp16 = mybir.dt.float16
i8 = mybir.dt.int8
Alu = mybir.AluOpType
Act = mybir.ActivationFunctionType
Axis = mybir.AxisListType

B, C, P, D, T = 16384, 100, 10, 256, 1000
NCORES = 8
BC = B // NCORES       # 2048 samples per core
NT = BC // 128         # 16 batch tiles per core
GROUPS = [2, 4, 6, 4]  # tiles per compute group (sum = NT)
HOST_G = 1             # trailing groups whose smax ships raw to host
TRV = T // NCORES      # 125 prototype rows per core
BAND = 140             # same-class band width (>= 130 needed)
PC = P * C             # free size per tile
MARGIN = 0.3
CLST_SCALE = 0.8
SEP_SCALE = 0.08
DIV_SCALE = 0.01
CONTRASTIVE_SCALE = 0.1

OHV = -128.0           # one-hot mask/weight value (int8-representable)
SEP_TH = 1.0 - MARGIN  # separation threshold

_PROGRAMS = {}


def _build():
    nc = bacc.Bacc("TRN2", target_bir_lowering=False, debug=False,
                   num_devices=NCORES)
    sims_d = nc.dram_tensor("sims", [128, NT * PC], fp16,
                            kind="ExternalInput").ap()
    ohm_d = nc.dram_tensor("ohm", [128, NT, C], i8, kind="ExternalInput").ap()
    grhs_d = nc.dram_tensor("grhs", [128, 2, BAND + 1], fp16,
                            kind="ExternalInput").ap()
    rt2_d = nc.dram_tensor("rt2", [128, 2, 128], fp16,
                           kind="ExternalInput").ap()
    mdiv_d = nc.dram_tensor("mdiv", [128, BAND], fp16,
                            kind="ExternalInput").ap()
    outcls_d = nc.dram_tensor("out_cls", [C, C + 2], fp32,
                              kind="ExternalOutput").ap()
    outpr_d = nc.dram_tensor("out_pr", [128, 2], fp32,
                             kind="ExternalOutput").ap()
    HT = sum(GROUPS[-HOST_G:])   # trailing tiles handled host-side
    outsm_d = nc.dram_tensor("out_sm", [128, HT * C], fp16,
                             kind="ExternalOutput").ap()
    outom_d = nc.dram_tensor("out_om", [128, NT - HT], fp16,
                             kind="ExternalOutput").ap()

    with tile.TileContext(nc) as tc:
        with (
            tc.tile_pool(name="consts", bufs=1) as consts,
            tc.tile_pool(name="gr", bufs=1) as grp_,
            tc.tile_pool(name="outp", bufs=1) as outp,
            tc.tile_pool(name="psA", bufs=1, space="PSUM") as psA,
            tc.tile_pool(name="psG", bufs=1, space="PSUM") as psG,
        ):
            M = consts.tile([128, NT, PC], fp16, tag="M")
            SM6 = consts.tile([128, HT, C], fp16, tag="SM6")
            W = consts.tile([128, NT, 500], fp16, tag="W")
            X = consts.tile([128, NT, 200], fp16, tag="X")
            Y = consts.tile([128, NT, 100], fp16, tag="Y")
            J = consts.tile([128, NT, C], fp16, tag="J")
            OH = consts.tile([128, NT, C], fp16, tag="OH")
            RH = consts.tile([128, NT, C + 2], fp16, tag="RH")
            OM = consts.tile([128, NT], fp16, tag="OM")
            nhalf = consts.tile([128, 1], fp32, tag="nhalf")
            nc.vector.memset(nhalf[:], -0.5)
            cls_ps = psA.tile([C, C], fp32, tag="cls")

            # one-hot (int8 -> fp16 cast on SWDGE; off the sims queue)
            nc.gpsimd.dma_start(OH[:], ohm_d[:], max_dma_last_dim=2000)

            # sims group DMAs on HWDGE, smallest group first
            bounds = np.cumsum([0] + GROUPS)
            for g, n in enumerate(GROUPS):
                g0, g1 = int(bounds[g]), int(bounds[g + 1])
                nc.sync.dma_start(M[:, g0:g1, :],
                                  sims_d[:, g0 * PC:g1 * PC])
            # gram inputs land behind the sims stream
            grhs_t = consts.tile([128, 2, BAND + 1], fp16, tag="grhs")
            rt2_t = consts.tile([128, 2, 128], fp16, tag="rt2")
            mdiv_t = consts.tile([128, BAND], fp16, tag="mdiv")
            nc.sync.dma_start(grhs_t[:], grhs_d[:])
            nc.sync.dma_start(rt2_t[:], rt2_d[:])
            nc.sync.dma_start(mdiv_t[:], mdiv_d[:])

            for g, n in enumerate(GROUPS):
                g0, g1 = int(bounds[g]), int(bounds[g + 1])
                sl = slice(g0, g1)
                # slot-max tree (slot-major: position q*C + c)
                nc.vector.tensor_tensor(W[:, sl, :], M[:, sl, 0:500],
                                        M[:, sl, 500:1000], op=Alu.max)
                nc.vector.tensor_tensor(X[:, sl, :], W[:, sl, 0:200],
                                        W[:, sl, 200:400], op=Alu.max)
                nc.vector.tensor_tensor(Y[:, sl, :], X[:, sl, 0:100],
                                        X[:, sl, 100:200], op=Alu.max)
                last = g0 >= NT - HT
                # trailing groups' smax goes to a compact buffer: their
                # cluster/sep/count sides are computed on host from the raw
                # smax (kills the j2/relu/matmul serial tail)
                h0, h1 = g0 - (NT - HT), g1 - (NT - HT)
                smax_dst = SM6[:, h0:h1, :] if last else RH[:, sl, 0:C]
                nc.vector.tensor_tensor(smax_dst, Y[:, sl, :],
                                        W[:, sl, 400:500], op=Alu.max)
                if last:
                    # nothing else depends on this group on-device: ship it
                    nc.sync.dma_start(outsm_d[:, h0 * C:h1 * C],
                                      SM6[:, h0:h1, :])
                else:
                    # per-class smax sums via PE (don't wait for sep)
                    for i in range(g0, g1):
                        nc.tensor.matmul(cls_ps[:], OH[:, i, :],
                                         RH[:, i, 0:C], start=(i == 0),
                                         stop=(i == NT - HT - 1))
                    if g1 == NT - HT:
                        # cluster sums complete: output issues here, its
                        # DRAM receipt hides under the last group's tree
                        ocl1 = outp.tile([C, C], fp32, tag="ocl1")
                        nc.scalar.copy(ocl1[:], cls_ps[:])
                        nc.sync.dma_start(outcls_d[:, 0:C], ocl1[:])
                        # gram: 2 matmuls + relu/mask/reduce (inputs landed)
                        g_ps = psG.tile([128, BAND + 1], fp32, tag="g")
                        for k in (0, 1):
                            nc.tensor.matmul(g_ps[:], rt2_t[:, k, :],
                                             grhs_t[:, k, :],
                                             start=(k == 0), stop=(k == 1))
                        rel = grp_.tile([128, BAND], fp16, tag="rel")
                        nc.scalar.activation(rel[:], g_ps[:, 0:BAND],
                                             Act.Relu, bias=nhalf[:])
                        junk = grp_.tile([128, BAND], fp16, tag="junk")
                        opr = outp.tile([128, 2], fp32, tag="opr")
                        nc.vector.tensor_tensor(junk[:], rel[:], mdiv_t[:],
                                                op=Alu.mult)
                        nc.vector.tensor_reduce(opr[:, 0:1], junk[:],
                                                axis=Axis.X, op=Alu.add)
                        nc.scalar.copy(opr[:, 1:2], g_ps[:, BAND:BAND + 1])
                        nc.sync.dma_start(outpr_d[:], opr[:])
                        continue  # its j2 is deferred past the last group
                    # other-class max (separation finished on host)
                    nc.vector.tensor_tensor(J[:, sl, :], RH[:, sl, 0:C],
                                            OH[:, sl, :], op=Alu.add)
                    nc.vector.tensor_reduce(OM[:, sl], J[:, sl, :],
                                            axis=Axis.X, op=Alu.max)

                if g == len(GROUPS) - 1:
                    # deferred j2 for the last leading group: runs after this
                    # group's tree so the outsm DMA issues as early as possible
                    d0, d1 = NT - HT - GROUPS[-HOST_G - 1], NT - HT
                    dsl = slice(d0, d1)
                    nc.vector.tensor_tensor(J[:, dsl, :], RH[:, dsl, 0:C],
                                            OH[:, dsl, :], op=Alu.add)
                    nc.vector.tensor_reduce(OM[:, dsl], J[:, dsl, :],
                                            axis=Axis.X, op=Alu.max)
                    nc.sync.dma_start(outom_d[:], OM[:, 0:NT - HT])

    nc.compile()
    return nc


def _get_program():
    if "v3" not in _PROGRAMS:
        _PROGRAMS["v3"] = _build()
    return _PROGRAMS["v3"]


def _numpy_fallback(similarities, labels, prototypes, proto_indices, valid_mask):
    """Pure-numpy replication of the reference (for unexpected shapes)."""
    s = similarities.astype(np.float64)
    Bx, Cx, Px = s.shape
    Tx = prototypes.shape[0]
    distances = 1.0 - s
    starts = proto_indices[:, 0]
    ends = proto_indices[:, 1]
    counts = ends - starts
    pvalid = np.arange(Px)[None, :] < counts[:, None]
    dmask = np.where(pvalid[None, :, :], distances, np.inf)
    min_all = dmask.min(axis=-1)
    own_min = min_all[np.arange(Bx), labels]
    cls_n = np.bincount(labels, minlength=Cx).astype(np.float64)
    cls_sum = np.bincount(labels, weights=own_min, minlength=Cx)
    has = cls_n > 0
    nvalid = max(int(has.sum()), 1)
    mean_c = cls_sum / np.maximum(cls_n, 1.0)
    w = 1.0 / np.sqrt(cls_n + 1e-6)
    cluster = np.where(has, w * mean_c, 0.0).sum() / nvalid * CLST_SCALE
    m2 = min_all.copy()
    m2[np.arange(Bx), labels] = np.inf
    other_min = m2.min(axis=-1)
    sep_term = np.maximum(MARGIN - other_min, 0.0)
    sep_cls = np.bincount(labels, weights=sep_term, minlength=Cx)
    sep = np.where(has, sep_cls / np.maximum(cls_n, 1.0), 0.0).sum() / nvalid * SEP_SCALE
    pr = prototypes.astype(np.float64)
    norm = np.sqrt((pr * pr).sum(-1, keepdims=True))
    pn = pr / np.maximum(norm, 1e-12)
    sim = pn @ pn.T
    proto_class = np.searchsorted(starts, np.arange(Tx), side="right") - 1
    same = proto_class[:, None] == proto_class[None, :]
    offd = ~np.eye(Tx, dtype=bool)
    pair = same & offd
    relv = np.maximum(sim - 0.5, 0.0)
    row_sum = np.where(pair, relv, 0.0).sum(1)
    cls_pair = np.bincount(proto_class, weights=row_sum, minlength=Cx)
    npairs = (counts * (counts - 1)).astype(np.float64)
    dvalid = counts > 1
    ndv = max(int(dvalid.sum()), 1)
    div = np.where(dvalid, cls_pair / np.maximum(npairs, 1.0), 0.0).sum() / ndv * DIV_SCALE
    vm = valid_mask.astype(bool)
    vpair = (vm[:, None] & vm[None, :]) & offd
    nvp = max(int(vpair.sum()), 1)
    contrast = np.where(vpair, sim, 0.0).sum() / nvp * CONTRASTIVE_SCALE
    total = cluster + sep + div + contrast
    return np.array([cluster, sep, div, contrast, total], dtype=np.float32)


def kernel(similarities, labels, prototypes, proto_indices, valid_mask,
           max_prototypes=None, **_ignored):
    similarities = np.asarray(similarities, dtype=np.float32)
    labels = np.asarray(labels)
    prototypes = np.asarray(prototypes, dtype=np.float32)
    proto_indices = np.asarray(proto_indices)
    valid_mask = np.asarray(valid_mask).astype(bool)

    starts = proto_indices[:, 0].astype(np.int64)
    ends = proto_indices[:, 1].astype(np.int64)
    counts = ends - starts

    if (similarities.shape != (B, C, P) or prototypes.shape != (T, D)
            or not bool((counts == P).all()) or not bool(valid_mask.all())):
        return _numpy_fallback(similarities, labels, prototypes,
                               proto_indices, valid_mask)

    labels_i = labels.astype(np.int64)

    # ---- sims -> fp16 slot-major [B, P, C] ----
    sq = similarities.astype(np.float16)
    X = np.ascontiguousarray(sq.transpose(0, 2, 1)).reshape(B, PC)

    # ---- gram host prep ----
    nrm = np.sqrt((prototypes * prototypes).sum(-1))
    pn16 = (prototypes / np.maximum(nrm, 1e-12)[:, None]).astype(np.float16)
    colsum = pn16.astype(np.float32).sum(0)       # [D]
    proto_class = np.arange(T) // P

    in_maps = []
    for c in range(NCORES):
        Xc = X[c * BC:(c + 1) * BC]               # [2048, PC]
        sims_np = np.ascontiguousarray(
            Xc.reshape(NT, 128, PC).transpose(1, 0, 2)).reshape(128, NT * PC)

        lab_c = labels_i[c * BC:(c + 1) * BC].reshape(NT, 128)
        ohm = np.zeros((128, NT, C), np.int8)
        ii, pp_ = np.meshgrid(np.arange(NT), np.arange(128), indexing="ij")
        ohm[pp_.ravel(), ii.ravel(), lab_c.ravel()] = int(OHV)

        r0 = c * TRV
        bs = (r0 // P) * P
        rows = np.arange(r0, r0 + 128)
        rin = rows < T
        rows_c = np.minimum(rows, T - 1)
        cols = np.arange(bs, bs + BAND)
        cin = cols < T
        cols_c = np.minimum(cols, T - 1)
        # rt2[d, k, r] = pn[r0+r, 128k+d]; grhs[d, k, j] = pn[bs+j, 128k+d]
        rslice = pn16[rows_c] * rin[:, None].astype(np.float16)   # [128, D]
        rt2 = np.ascontiguousarray(
            rslice.reshape(128, 2, 128).transpose(2, 1, 0))       # [128d,2,128r]
        bslice = pn16[cols_c] * cin[:, None].astype(np.float16)   # [BAND, D]
        grhs = np.zeros((128, 2, BAND + 1), np.float16)
        grhs[:, :, 0:BAND] = bslice.reshape(BAND, 2, 128).transpose(2, 1, 0)
        grhs[:, :, BAND] = colsum.reshape(2, 128).transpose(1, 0)
        md = (proto_class[rows_c][:, None] == proto_class[cols_c][None, :])
        md &= rows_c[:, None] != cols_c[None, :]
        md &= rin[:, None] & cin[None, :]
        md[TRV:] = False
        mdiv = md.astype(np.float16)

        in_maps.append(dict(sims=sims_np, ohm=ohm, grhs=grhs, rt2=rt2,
                            mdiv=mdiv))

    nc = _get_program()
    res = run_bass_kernel_spmd(nc, in_maps, core_ids=list(range(NCORES)))
    results = res.results

    f32 = np.float32
    cls = np.sum(np.stack([results[c]["out_cls"] for c in range(NCORES)]),
                 axis=0, dtype=np.float32) / f32(OHV)   # [100, 102] true sums
    own_smax_sum = np.diag(cls[:, 0:C]).astype(f32)
    sep_cls_sum = np.zeros(C, np.float32)
    cls_n = np.bincount(labels_i, minlength=C).astype(f32)

    # sep for leading tiles from the shipped other-class max
    HT0 = sum(GROUPS[-HOST_G:])
    for c in range(NCORES):
        om = results[c]["out_om"].astype(np.float32)       # [128, NT-HT0]
        labL = labels_i[c * BC:c * BC + (NT - HT0) * 128].reshape(NT - HT0, 128)
        labL = np.ascontiguousarray(labL.T)                # [128, NT-HT0]
        sep_t = np.maximum(om - f32(SEP_TH), f32(0.0))
        np.add.at(sep_cls_sum, labL.ravel(), sep_t.ravel().astype(f32))

    # cluster/sep/counts for trailing samples (raw smax shipped back)
    HT = sum(GROUPS[-HOST_G:])
    t0 = NT - HT
    for c in range(NCORES):
        sm = results[c]["out_sm"].reshape(128, HT, C).astype(np.float32)
        lab = labels_i[c * BC + t0 * 128:(c + 1) * BC].reshape(HT, 128)
        lab = np.ascontiguousarray(lab.T)                  # [128, HT]
        pidx = np.arange(128)[:, None], np.arange(HT)[None, :]
        own = sm[pidx[0], pidx[1], lab]
        np.add.at(own_smax_sum, lab.ravel(), own.ravel().astype(f32))
        m = sm.copy()
        m[pidx[0], pidx[1], lab] = -np.inf
        sep_t = np.maximum(m.max(-1) - f32(SEP_TH), f32(0.0))
        np.add.at(sep_cls_sum, lab.ravel(), sep_t.ravel().astype(f32))

    has = cls_n > 0
    nvalid = f32(max(int(has.sum()), 1))
    own_min_sum = cls_n - own_smax_sum
    mean_c = (own_min_sum / np.maximum(cls_n, f32(1.0))).astype(f32)
    w = (f32(1.0) / np.sqrt(cls_n + f32(1e-6))).astype(f32)
    cluster = f32(np.where(has, w * mean_c, f32(0.0)).sum(dtype=np.float32)
                  / nvalid * f32(CLST_SCALE))
    sep = f32(np.where(has, sep_cls_sum / np.maximum(cls_n, f32(1.0)), f32(0.0))
              .sum(dtype=np.float32) / nvalid * f32(SEP_SCALE))

    divrow = np.concatenate([results[c]["out_pr"][:TRV, 0] for c in range(NCORES)])
    conrow = np.concatenate([results[c]["out_pr"][:TRV, 1] for c in range(NCORES)])
    cls_pair = np.zeros(C, np.float32)
    np.add.at(cls_pair, proto_class, divrow)
    npairs = (counts * (counts - 1)).astype(np.float32)
    dvalid = counts > 1
    ndv = f32(max(int(dvalid.sum()), 1))
    div = f32(np.where(dvalid, cls_pair / np.maximum(npairs, f32(1.0)), f32(0.0))
              .sum(dtype=np.float32) / ndv * f32(DIV_SCALE))

    svm = int(valid_mask.sum())
    nvp = f32(max(svm * svm - svm, 1))
    contrast = f32((conrow.sum(dtype=np.float32) - f32(T))
                   / nvp * f32(CONTRASTIVE_SCALE))

    total = f32(cluster + sep + div + contrast)
    return np.array([cluster, sep, div, contrast, total], dtype=np.float32)


# revision 63
# speedup vs baseline: 1.1253x; 1.1253x over previous
"""BalancedPrototypeLoss on 8 Trainium2 NeuronCores — v4.

Strategy (data-parallel over batch, band-parallel over prototypes):
  - similarities [16384,100,10] are cast to fp16 on host, reorganized to
    slot-major [128, NT, 10, 100] per core, and streamed over HWDGE in
    asymmetric groups (GROUPS tiles each) so the Vector engine starts early.
  - per group, the slot-max is a batched fp16 tensor_tensor max tree
    (2x DVE mode; tensor_reduce would be capped at 1x): 1000 -> 500 -> 200
    -> 100 per tile, all tiles of a group in one instruction.
  - for the leading groups: smax plus a -128 one-hot (int8 upload, SWDGE
    casts to fp16) is group-max-reduced for the separation term; per-class
    sums of smax and [sep | 1] accumulate on the TensorEngine as one-hot
    matmuls in two PSUM banks, copied out mid-kernel so the DRAM-write
    receipts hide under later compute.
  - the trailing HOST_G groups ship their raw smax to the host, which adds
    their cluster/sep/count contributions directly — the device tail is just
    the last max tree plus one DMA (no j2/relu/matmul serial chain).
  - prototype Gram: host normalizes+transposes prototypes; each core computes
    its 128-row slice against a 140-wide same-class band plus one column
    against colsum(pn) (exact Gram row sums) via 2 matmuls.
  - host combines per-core partials in float32.
"""

import sys

_TRN_REPO = "/opt/trn_rl_repo"
if _TRN_REPO not in sys.path:
    sys.path.insert(0, _TRN_REPO)

import numpy as np

import concourse.bacc as bacc
import concourse.mybir as mybir
from concourse import tile
from concourse.bass_utils import run_bass_kernel_spmd

fp32 = mybir.dt.float32
fp16 = mybir.dt.float16
i8 = mybir.dt.int8
Alu = mybir.AluOpType
Act = mybir.ActivationFunctionType
Axis = mybir.AxisListType

B, C, P, D, T = 16384, 100, 10, 256, 1000
NCORES = 8
BC = B // NCORES       # 2048 samples per core
NT = BC // 128         # 16 batch tiles per core
GROUPS = [2, 4, 6, 4]  # tiles per compute group (sum = NT)
HOST_G = 1             # trailing groups whose smax ships raw to host
TRV = T // NCORES      # 125 prototype rows per core
BAND = 140             # same-class band width (>= 130 needed)
PC = P * C             # free size per tile
MARGIN = 0.3
CLST_SCALE = 0.8
SEP_SCALE = 0.08
DIV_SCALE = 0.01
CONTRASTIVE_SCALE = 0.1

OHV = -128.0           # one-hot mask/weight value (int8-representable)
SEP_TH = 1.0 - MARGIN  # separation threshold

_PROGRAMS = {}


def _build():
    nc = bacc.Bacc("TRN2", target_bir_lowering=False, debug=False,
                   num_devices=NCORES)
    sims_d = nc.dram_tensor("sims", [128, NT * PC], fp16,
                            kind="ExternalInput").ap()
    ohm_d = nc.dram_tensor("ohm", [128, NT, C], i8, kind="ExternalInput").ap()
    grhs_d = nc.dram_tensor("grhs", [128, 2, BAND + 1], fp16,
                            kind="ExternalInput").ap()
    rt2_d = nc.dram_tensor("rt2", [128, 2, 128], fp16,
                           kind="ExternalInput").ap()
    mdiv_d = nc.dram_tensor("mdiv", [128, BAND], fp16,
                            kind="ExternalInput").ap()
    outcls_d = nc.dram_tensor("out_cls", [C, C + 2], fp32,
                              kind="ExternalOutput").ap()
    outpr_d = nc.dram_tensor("out_pr", [128, 2], fp32,
                             kind="ExternalOutput").ap()
    HT = sum(GROUPS[-HOST_G:])   # trailing tiles handled host-side
    outsm_d = nc.dram_tensor("out_sm", [128, HT * C], fp16,
                             kind="ExternalOutput").ap()
    outom_d = nc.dram_tensor("out_om", [128, NT - HT], fp16,
                             kind="ExternalOutput").ap()

    with tile.TileContext(nc) as tc:
        with (
            tc.tile_pool(name="consts", bufs=1) as consts,
            tc.tile_pool(name="gr", bufs=1) as grp_,
            tc.tile_pool(name="outp", bufs=1) as outp,
            tc.tile_pool(name="psA", bufs=1, space="PSUM") as psA,
            tc.tile_pool(name="psG", bufs=1, space="PSUM") as psG,
        ):
            M = consts.tile([128, NT, PC], fp16, tag="M")
            SM6 = consts.tile([128, HT, C], fp16, tag="SM6")
            W = consts.tile([128, NT, 500], fp16, tag="W")
            X = consts.tile([128, NT, 200], fp16, tag="X")
            Y = consts.tile([128, NT, 100], fp16, tag="Y")
            J = consts.tile([128, NT, C], fp16, tag="J")
            OH = consts.tile([128, NT, C], fp16, tag="OH")
            RH = consts.tile([128, NT, C + 2], fp16, tag="RH")
            OM = consts.tile([128, NT], fp16, tag="OM")
            nhalf = consts.tile([128, 1], fp32, tag="nhalf")
            nc.vector.memset(nhalf[:], -0.5)
            cls_ps = psA.tile([C, C], fp32, tag="cls")

            # one-hot (int8 -> fp16 cast on SWDGE; off the sims queue)
            nc.gpsimd.dma_start(OH[:], ohm_d[:], max_dma_last_dim=2000)

            # sims group DMAs on HWDGE, smallest group first
            bounds = np.cumsum([0] + GROUPS)
            for g, n in enumerate(GROUPS):
                g0, g1 = int(bounds[g]), int(bounds[g + 1])
                nc.sync.dma_start(M[:, g0:g1, :],
                                  sims_d[:, g0 * PC:g1 * PC])
            # gram inputs land behind the sims stream
            grhs_t = consts.tile([128, 2, BAND + 1], fp16, tag="grhs")
            rt2_t = consts.tile([128, 2, 128], fp16, tag="rt2")
            mdiv_t = consts.tile([128, BAND], fp16, tag="mdiv")
            nc.sync.dma_start(grhs_t[:], grhs_d[:])
            nc.sync.dma_start(rt2_t[:], rt2_d[:])
            nc.sync.dma_start(mdiv_t[:], mdiv_d[:])

            for g, n in enumerate(GROUPS):
                g0, g1 = int(bounds[g]), int(bounds[g + 1])
                sl = slice(g0, g1)
                # slot-max tree (slot-major: position q*C + c)
                nc.vector.tensor_tensor(W[:, sl, :], M[:, sl, 0:500],
                                        M[:, sl, 500:1000], op=Alu.max)
                nc.vector.tensor_tensor(X[:, sl, :], W[:, sl, 0:200],
                                        W[:, sl, 200:400], op=Alu.max)
                nc.vector.tensor_tensor(Y[:, sl, :], X[:, sl, 0:100],
                                        X[:, sl, 100:200], op=Alu.max)
                last = g0 >= NT - HT
                # trailing groups' smax goes to a compact buffer: their
                # cluster/sep/count sides are computed on host from the raw
                # smax (kills the j2/relu/matmul serial tail)
                h0, h1 = g0 - (NT - HT), g1 - (NT - HT)
                smax_dst = SM6[:, h0:h1, :] if last else RH[:, sl, 0:C]
                nc.vector.tensor_tensor(smax_dst, Y[:, sl, :],
                                        W[:, sl, 400:500], op=Alu.max)
                if last:
                    # nothing else depends on this group on-device: ship it
                    nc.sync.dma_start(outsm_d[:, h0 * C:h1 * C],
                                      SM6[:, h0:h1, :])
                else:
                    # per-class smax sums via PE (don't wait for sep)
                    for i in range(g0, g1):
                        nc.tensor.matmul(cls_ps[:], OH[:, i, :],
                                         RH[:, i, 0:C], start=(i == 0),
                                         stop=(i == NT - HT - 1))
                    if g1 == NT - HT:
                        # cluster sums complete: output issues here, its
                        # DRAM receipt hides under the last group's tree
                        ocl1 = outp.tile([C, C], fp32, tag="ocl1")
                        nc.scalar.copy(ocl1[:], cls_ps[:])
                        nc.sync.dma_start(outcls_d[:, 0:C], ocl1[:])
                        continue  # its j2 is deferred past the last group
                    # other-class max (separation finished on host)
                    nc.vector.tensor_tensor(J[:, sl, :], RH[:, sl, 0:C],
                                            OH[:, sl, :], op=Alu.add)
                    nc.vector.tensor_reduce(OM[:, sl], J[:, sl, :],
                                            axis=Axis.X, op=Alu.max)

                if g == len(GROUPS) - 1:
                    # deferred j2 for the last leading group: runs after this
                    # group's tree so the outsm DMA issues as early as possible
                    d0, d1 = NT - HT - GROUPS[-HOST_G - 1], NT - HT
                    dsl = slice(d0, d1)
                    nc.vector.tensor_tensor(J[:, dsl, :], RH[:, dsl, 0:C],
                                            OH[:, dsl, :], op=Alu.add)
                    nc.vector.tensor_reduce(OM[:, dsl], J[:, dsl, :],
                                            axis=Axis.X, op=Alu.max)
                    nc.sync.dma_start(outom_d[:], OM[:, 0:NT - HT])
                    # gram: 2 matmuls + relu/mask/reduce, on the tail where
                    # its late-landing inputs can't block earlier tree work
                    g_ps = psG.tile([128, BAND + 1], fp32, tag="g")
                    for k in (0, 1):
                        nc.tensor.matmul(g_ps[:], rt2_t[:, k, :],
                                         grhs_t[:, k, :],
                                         start=(k == 0), stop=(k == 1))
                    rel = grp_.tile([128, BAND], fp16, tag="rel")
                    nc.scalar.activation(rel[:], g_ps[:, 0:BAND],
                                         Act.Relu, bias=nhalf[:])
                    junk = grp_.tile([128, BAND], fp16, tag="junk")
                    opr = outp.tile([128, 2], fp32, tag="opr")
                    nc.vector.tensor_tensor(junk[:], rel[:], mdiv_t[:],
                                            op=Alu.mult)
                    nc.vector.tensor_reduce(opr[:, 0:1], junk[:],
                                            axis=Axis.X, op=Alu.add)
                    nc.scalar.copy(opr[:, 1:2], g_ps[:, BAND:BAND + 1])
                    nc.sync.dma_start(outpr_d[:], opr[:])

    nc.compile()
    return nc


def _get_program():
    if "v3" not in _PROGRAMS:
        _PROGRAMS["v3"] = _build()
    return _PROGRAMS["v3"]


def _numpy_fallback(similarities, labels, prototypes, proto_indices, valid_mask):
    """Pure-numpy replication of the reference (for unexpected shapes)."""
    s = similarities.astype(np.float64)
    Bx, Cx, Px = s.shape
    Tx = prototypes.shape[0]
    distances = 1.0 - s
    starts = proto_indices[:, 0]
    ends = proto_indices[:, 1]
    counts = ends - starts
    pvalid = np.arange(Px)[None, :] < counts[:, None]
    dmask = np.where(pvalid[None, :, :], distances, np.inf)
    min_all = dmask.min(axis=-1)
    own_min = min_all[np.arange(Bx), labels]
    cls_n = np.bincount(labels, minlength=Cx).astype(np.float64)
    cls_sum = np.bincount(labels, weights=own_min, minlength=Cx)
    has = cls_n > 0
    nvalid = max(int(has.sum()), 1)
    mean_c = cls_sum / np.maximum(cls_n, 1.0)
    w = 1.0 / np.sqrt(cls_n + 1e-6)
    cluster = np.where(has, w * mean_c, 0.0).sum() / nvalid * CLST_SCALE
    m2 = min_all.copy()
    m2[np.arange(Bx), labels] = np.inf
    other_min = m2.min(axis=-1)
    sep_term = np.maximum(MARGIN - other_min, 0.0)
    sep_cls = np.bincount(labels, weights=sep_term, minlength=Cx)
    sep = np.where(has, sep_cls / np.maximum(cls_n, 1.0), 0.0).sum() / nvalid * SEP_SCALE
    pr = prototypes.astype(np.float64)
    norm = np.sqrt((pr * pr).sum(-1, keepdims=True))
    pn = pr / np.maximum(norm, 1e-12)
    sim = pn @ pn.T
    proto_class = np.searchsorted(starts, np.arange(Tx), side="right") - 1
    same = proto_class[:, None] == proto_class[None, :]
    offd = ~np.eye(Tx, dtype=bool)
    pair = same & offd
    relv = np.maximum(sim - 0.5, 0.0)
    row_sum = np.where(pair, relv, 0.0).sum(1)
    cls_pair = np.bincount(proto_class, weights=row_sum, minlength=Cx)
    npairs = (counts * (counts - 1)).astype(np.float64)
    dvalid = counts > 1
    ndv = max(int(dvalid.sum()), 1)
    div = np.where(dvalid, cls_pair / np.maximum(npairs, 1.0), 0.0).sum() / ndv * DIV_SCALE
    vm = valid_mask.astype(bool)
    vpair = (vm[:, None] & vm[None, :]) & offd
    nvp = max(int(vpair.sum()), 1)
    contrast = np.where(vpair, sim, 0.0).sum() / nvp * CONTRASTIVE_SCALE
    total = cluster + sep + div + contrast
    return np.array([cluster, sep, div, contrast, total], dtype=np.float32)


def kernel(similarities, labels, prototypes, proto_indices, valid_mask,
           max_prototypes=None, **_ignored):
    similarities = np.asarray(similarities, dtype=np.float32)
    labels = np.asarray(labels)
    prototypes = np.asarray(prototypes, dtype=np.float32)
    proto_indices = np.asarray(proto_indices)
    valid_mask = np.asarray(valid_mask).astype(bool)

    starts = proto_indices[:, 0].astype(np.int64)
    ends = proto_indices[:, 1].astype(np.int64)
    counts = ends - starts

    if (similarities.shape != (B, C, P) or prototypes.shape != (T, D)
            or not bool((counts == P).all()) or not bool(valid_mask.all())):
        return _numpy_fallback(similarities, labels, prototypes,
                               proto_indices, valid_mask)

    labels_i = labels.astype(np.int64)

    # ---- sims -> fp16 slot-major [B, P, C] ----
    sq = similarities.astype(np.float16)
    X = np.ascontiguousarray(sq.transpose(0, 2, 1)).reshape(B, PC)

    # ---- gram host prep ----
    nrm = np.sqrt((prototypes * prototypes).sum(-1))
    pn16 = (prototypes / np.maximum(nrm, 1e-12)[:, None]).astype(np.float16)
    colsum = pn16.astype(np.float32).sum(0)       # [D]
    proto_class = np.arange(T) // P

    in_maps = []
    for c in range(NCORES):
        Xc = X[c * BC:(c + 1) * BC]               # [2048, PC]
        sims_np = np.ascontiguousarray(
            Xc.reshape(NT, 128, PC).transpose(1, 0, 2)).reshape(128, NT * PC)

        lab_c = labels_i[c * BC:(c + 1) * BC].reshape(NT, 128)
        ohm = np.zeros((128, NT, C), np.int8)
        ii, pp_ = np.meshgrid(np.arange(NT), np.arange(128), indexing="ij")
        ohm[pp_.ravel(), ii.ravel(), lab_c.ravel()] = int(OHV)

        r0 = c * TRV
        bs = (r0 // P) * P
        rows = np.arange(r0, r0 + 128)
        rin = rows < T
        rows_c = np.minimum(rows, T - 1)
        cols = np.arange(bs, bs + BAND)
        cin = cols < T
        cols_c = np.minimum(cols, T - 1)
        # rt2[d, k, r] = pn[r0+r, 128k+d]; grhs[d, k, j] = pn[bs+j, 128k+d]
        rslice = pn16[rows_c] * rin[:, None].astype(np.float16)   # [128, D]
        rt2 = np.ascontiguousarray(
            rslice.reshape(128, 2, 128).transpose(2, 1, 0))       # [128d,2,128r]
        bslice = pn16[cols_c] * cin[:, None].astype(np.float16)   # [BAND, D]
        grhs = np.zeros((128, 2, BAND + 1), np.float16)
        grhs[:, :, 0:BAND] = bslice.reshape(BAND, 2, 128).transpose(2, 1, 0)
        grhs[:, :, BAND] = colsum.reshape(2, 128).transpose(1, 0)
        md = (proto_class[rows_c][:, None] == proto_class[cols_c][None, :])
        md &= rows_c[:, None] != cols_c[None, :]
        md &= rin[:, None] & cin[None, :]
        md[TRV:] = False
        mdiv = md.astype(np.float16)

        in_maps.append(dict(sims=sims_np, ohm=ohm, grhs=grhs, rt2=rt2,
                            mdiv=mdiv))

    nc = _get_program()
    res = run_bass_kernel_spmd(nc, in_maps, core_ids=list(range(NCORES)))
    results = res.results

    f32 = np.float32
    cls = np.sum(np.stack([results[c]["out_cls"] for c in range(NCORES)]),
                 axis=0, dtype=np.float32) / f32(OHV)   # [100, 102] true sums
    own_smax_sum = np.diag(cls[:, 0:C]).astype(f32)
    sep_cls_sum = np.zeros(C, np.float32)
    cls_n = np.bincount(labels_i, minlength=C).astype(f32)

    # sep for leading tiles from the shipped other-class max
    HT0 = sum(GROUPS[-HOST_G:])
    for c in range(NCORES):
        om = results[c]["out_om"].astype(np.float32)       # [128, NT-HT0]
        labL = labels_i[c * BC:c * BC + (NT - HT0) * 128].reshape(NT - HT0, 128)
        labL = np.ascontiguousarray(labL.T)                # [128, NT-HT0]
        sep_t = np.maximum(om - f32(SEP_TH), f32(0.0))
        np.add.at(sep_cls_sum, labL.ravel(), sep_t.ravel().astype(f32))

    # cluster/sep/counts for trailing samples (raw smax shipped back)
    HT = sum(GROUPS[-HOST_G:])
    t0 = NT - HT
    for c in range(NCORES):
        sm = results[c]["out_sm"].reshape(128, HT, C).astype(np.float32)
        lab = labels_i[c * BC + t0 * 128:(c + 1) * BC].reshape(HT, 128)
        lab = np.ascontiguousarray(lab.T)                  # [128, HT]
        pidx = np.arange(128)[:, None], np.arange(HT)[None, :]
        own = sm[pidx[0], pidx[1], lab]
        np.add.at(own_smax_sum, lab.ravel(), own.ravel().astype(f32))
        m = sm.copy()
        m[pidx[0], pidx[1], lab] = -np.inf
        sep_t = np.maximum(m.max(-1) - f32(SEP_TH), f32(0.0))
        np.add.at(sep_cls_sum, lab.ravel(), sep_t.ravel().astype(f32))

    has = cls_n > 0
    nvalid = f32(max(int(has.sum()), 1))
    own_min_sum = cls_n - own_smax_sum
    mean_c = (own_min_sum / np.maximum(cls_n, f32(1.0))).astype(f32)
    w = (f32(1.0) / np.sqrt(cls_n + f32(1e-6))).astype(f32)
    cluster = f32(np.where(has, w * mean_c, f32(0.0)).sum(dtype=np.float32)
                  / nvalid * f32(CLST_SCALE))
    sep = f32(np.where(has, sep_cls_sum / np.maximum(cls_n, f32(1.0)), f32(0.0))
              .sum(dtype=np.float32) / nvalid * f32(SEP_SCALE))

    divrow = np.concatenate([results[c]["out_pr"][:TRV, 0] for c in range(NCORES)])
    conrow = np.concatenate([results[c]["out_pr"][:TRV, 1] for c in range(NCORES)])
    cls_pair = np.zeros(C, np.float32)
    np.add.at(cls_pair, proto_class, divrow)
    npairs = (counts * (counts - 1)).astype(np.float32)
    dvalid = counts > 1
    ndv = f32(max(int(dvalid.sum()), 1))
    div = f32(np.where(dvalid, cls_pair / np.maximum(npairs, f32(1.0)), f32(0.0))
              .sum(dtype=np.float32) / ndv * f32(DIV_SCALE))

    svm = int(valid_mask.sum())
    nvp = f32(max(svm * svm - svm, 1))
    contrast = f32((conrow.sum(dtype=np.float32) - f32(T))
                   / nvp * f32(CONTRASTIVE_SCALE))

    total = f32(cluster + sep + div + contrast)
    return np.array([cluster, sep, div, contrast, total], dtype=np.float32)
